# revision 1
# baseline (speedup 1.0000x reference)
"""GQA attention kernel for Trainium2, 8 NeuronCores.

Problem: B=2, T=2048, D=1024, 16 Q heads / 4 KV heads, head_dim=64, RoPE,
causal softmax, out-projection.

Sharding: 8 cores = 2 (batch) x 4 (KV group). Core c handles batch c//4 and
KV group g=c%4 (query heads 4g..4g+3). wq/wk/wv column-sharded, wo
row-sharded; the 4 partial outputs per batch are summed on the host.

On-chip layout: everything is kept transposed (head_dim on partitions):
  xT (D, T), qT (256, T), kT (64, T).  Scores are computed directly in
transposed orientation scoresT[j, i] = k_j . q_i (j on partitions), so no
on-chip transposes of the attention matrix are needed.  Softmax runs without
max-subtraction (scores are O(6) bounded), and the denominator L[i] is
obtained for free by augmenting V with a ones-column in the PV matmul.
RoPE pairs are de-interleaved via a host-side column permutation of wq/wk so
rotate-half applies; the interleave never needs to be undone because q and k
share the same permutation and V/out stay in natural order.

All matmuls run as float32r (full fp32 data, fast PE mode).  Engines have no
cross-partition paths, so every partition-base change (rotate-half swap, kT
duplication, odd-head placement) goes through SBUF->SBUF DMA.
"""

import numpy as np
import sys

sys.path.insert(0, "/opt/trn_rl_repo")

from concourse import bass, bacc, mybir, tile  # noqa: E402
from concourse.bass_utils import run_bass_kernel_spmd  # noqa: E402

F32 = mybir.dt.float32
F32R = mybir.dt.float32r

B, T, D = 2, 2048, 1024
HD = 64                      # head dim
NQH = 4                      # query heads per core
QCOLS = NQH * HD             # 256
KC = D // 128                # 8 contraction chunks
NT = T // 128                # 16 row tiles
NC4 = T // 512               # 4 512-wide column chunks
N_CORES = 8

_cache = {}


def _r(ap):
    return ap.bitcast(F32R)


def build_nc():
    """Build the (SPMD-identical) single-core bass program."""
    nc = bacc.Bacc("TRN2", target_bir_lowering=False, debug=False)

    xT_d = nc.declare_dram_parameter("xT", [D, T], F32R, isOutput=False)
    wq_d = nc.declare_dram_parameter("wq", [D, QCOLS], F32R, isOutput=False)
    wk_d = nc.declare_dram_parameter("wk", [D, HD], F32R, isOutput=False)
    wv_d = nc.declare_dram_parameter("wv", [D, HD], F32R, isOutput=False)
    wo_d = nc.declare_dram_parameter("wo", [QCOLS, D], F32R, isOutput=False)
    cos_d = nc.declare_dram_parameter("cosf", [128, T], F32, isOutput=False)
    sin_d = nc.declare_dram_parameter("sinf", [128, T], F32, isOutput=False)
    msk_d = nc.declare_dram_parameter("msk", [128, 4, 512], F32, isOutput=False)
    one_d = nc.declare_dram_parameter("onec", [128, HD], F32R, isOutput=False)
    out_d = nc.declare_dram_parameter("out", [T, D], F32, isOutput=True)

    with tile.TileContext(nc) as tc:
        with tc.tile_pool(name="sb", bufs=1) as sb:
            wq = sb.tile([128, KC, QCOLS], F32, tag="wq")
            wk = sb.tile([128, KC, HD], F32, tag="wk")
            wv = sb.tile([128, KC, HD], F32, tag="wv")
            wo = sb.tile([128, 2, D], F32, tag="wo")
            cosf = sb.tile([128, T], F32, tag="cosf")
            sinf = sb.tile([128, T], F32, tag="sinf")
            msk = sb.tile([128, 4, 512], F32, tag="msk")
            # ones row placed at partition 64 to align with the L row of the
            # PV accumulator (engines need matching partition bases).
            ones = sb.tile([65, HD], F32, tag="ones")
            qT = [sb.tile([128, T], F32, tag=f"qT{hp}", name=f"qT{hp}")
                  for hp in range(2)]
            # kT duplicated into both partition halves so scores matmuls can
            # read it at base partition 0 (even heads) or 64 (odd heads).
            kT = sb.tile([128, T], F32, tag="kT")
            v = sb.tile([128, NT, HD + 1], F32, tag="v")
            ao = [sb.tile([128, T], F32, tag=f"ao{hp}", name=f"ao{hp}")
                  for hp in range(2)]

            for k in range(KC):
                nc.sync.dma_start(_r(wq[:, k, :]), wq_d[k * 128:(k + 1) * 128, :])
                nc.sync.dma_start(_r(wk[:, k, :]), wk_d[k * 128:(k + 1) * 128, :])
                nc.sync.dma_start(_r(wv[:, k, :]), wv_d[k * 128:(k + 1) * 128, :])
            nc.sync.dma_start(cosf[:], cos_d[:])
            nc.sync.dma_start(sinf[:], sin_d[:])
            nc.sync.dma_start(msk[:], msk_d[:])
            for c in range(2):
                nc.sync.dma_start(_r(wo[:, c, :]), wo_d[c * 128:(c + 1) * 128, :])

            nc.sync.dma_start(_r(ones[64:65, :]), one_d[64:65, :])
            nc.sync.dma_start(_r(v[:, :, HD:HD + 1]), one_d[:, 0:NT])

            # --- projections (xT lives only here) ---
            with (
                tc.tile_pool(name="sbx", bufs=1) as sbx,
                tc.tile_pool(name="rope", bufs=1) as rope_pool,
                tc.tile_pool(name="ppsum", bufs=2, space="PSUM") as ppsum,
            ):
                xT = sbx.tile([128, KC, T], F32, tag="xT")
                for k in range(KC):
                    nc.sync.dma_start(_r(xT[:, k, :]), xT_d[k * 128:(k + 1) * 128, :])

                def rope_inplace(q_ap, nrows):
                    """q = q*cos + rot_half(q)*sin, on de-interleaved rows."""
                    rot = rope_pool.tile([128, T], F32, tag="rot")
                    for blk in range(nrows // 64):
                        r0 = blk * 64
                        nc.sync.dma_start(rot[r0:r0 + 32, :],
                                          q_ap[r0 + 32:r0 + 64, :])
                        nc.sync.dma_start(rot[r0 + 32:r0 + 64, :],
                                          q_ap[r0:r0 + 32, :])
                    nc.vector.tensor_mul(_r(q_ap[0:nrows, :]), q_ap[0:nrows, :],
                                         cosf[0:nrows, :])
                    nc.vector.tensor_mul(rot[0:nrows, :], rot[0:nrows, :],
                                         sinf[0:nrows, :])
                    nc.vector.tensor_add(_r(q_ap[0:nrows, :]), q_ap[0:nrows, :],
                                         rot[0:nrows, :])

                for hp in range(2):
                    pq = ppsum.tile([128, T], F32, tag="proj")
                    for ci in range(NC4):
                        cs = slice(ci * 512, (ci + 1) * 512)
                        for k in range(KC):
                            nc.tensor.matmul(
                                pq[:, cs],
                                _r(wq[:, k, hp * 128:(hp + 1) * 128]),
                                _r(xT[:, k, cs]),
                                start=(k == 0), stop=(k == KC - 1))
                    nc.scalar.copy(_r(qT[hp][:]), pq[:])
                    rope_inplace(qT[hp][:], 128)

                pk = ppsum.tile([64, T], F32, tag="proj")
                for ci in range(NC4):
                    cs = slice(ci * 512, (ci + 1) * 512)
                    for k in range(KC):
                        nc.tensor.matmul(
                            pk[:, cs], _r(wk[:, k, :]), _r(xT[:, k, cs]),
                            start=(k == 0), stop=(k == KC - 1))
                nc.scalar.copy(_r(kT[0:64, :]), pk[:])
                rope_inplace(kT[:], 64)
                nc.sync.dma_start(_r(kT[64:128, :]), _r(kT[0:64, :]))

                for t in range(NT):
                    pv = ppsum.tile([128, HD], F32, tag="proj")
                    for k in range(KC):
                        nc.tensor.matmul(
                            pv[:], _r(xT[:, k, t * 128:(t + 1) * 128]),
                            _r(wv[:, k, :]),
                            start=(k == 0), stop=(k == KC - 1))
                    nc.scalar.copy(_r(v[:, t, 0:HD]), pv[:])

            # --- attention, one KV head (4 query heads) ---
            with (
                tc.tile_pool(name="aox", bufs=2) as aox,
                tc.tile_pool(name="at", bufs=6) as at_pool,
                tc.tile_pool(name="pvpsum", bufs=1, space="PSUM") as pvp,
                tc.tile_pool(name="scpsum", bufs=3, space="PSUM") as scp,
            ):
                for h in range(NQH):
                    hp, hr = divmod(h, 2)
                    qrow = slice(hr * 64, hr * 64 + 64)
                    pv_acc = pvp.tile([HD + 1, T], F32, tag="pv")
                    for ci in range(NC4):
                        cs = slice(ci * 512, (ci + 1) * 512)
                        n_tj = (ci + 1) * 4
                        for tj in range(n_tj):
                            sc = scp.tile([128, 512], F32, tag="sc")
                            nc.tensor.matmul(
                                sc[:],
                                _r(kT[qrow, tj * 128:(tj + 1) * 128]),
                                _r(qT[hp][qrow, cs]),
                                start=True, stop=True)
                            if tj >= ci * 4:  # diagonal block: causal mask
                                nc.vector.tensor_add(
                                    sc[:], sc[:], msk[:, tj - ci * 4, :])
                            at = at_pool.tile([128, 512], F32, tag="at")
                            nc.scalar.activation(
                                _r(at[:]), sc[:],
                                mybir.ActivationFunctionType.Exp,
                                scale=0.125)
                            nc.tensor.matmul(
                                pv_acc[:, cs], _r(v[:, tj, :]), _r(at[:]),
                                start=(tj == 0), stop=(tj == n_tj - 1))
                    # normalize: ao rows of head h = pv_acc[0:64] * (1/L);
                    # L sits in pv_acc row 64 (the ones-column of v_aug).
                    linv = aox.tile([65, T], F32, tag="linv")
                    with nc.allow_low_precision(reason="fp32r linv"):
                        nc.vector.reciprocal(_r(linv[64:65, :]),
                                             pv_acc[HD:HD + 1, :])
                    if hr == 0:
                        dst = ao[hp][0:64, :]
                    else:
                        dst = aox.tile([64, T], F32, tag="aotmp")
                    nc.scalar.copy(_r(dst), pv_acc[0:HD, :])
                    for ci in range(NC4):
                        cs = slice(ci * 512, (ci + 1) * 512)
                        lb = scp.tile([HD, 512], F32, tag="sc")
                        nc.tensor.matmul(lb[:], _r(ones[64:65, :]),
                                         _r(linv[64:65, cs]),
                                         start=True, stop=True)
                        nc.vector.tensor_mul(_r(dst[:, cs]), dst[:, cs], lb[:])
                    if hr == 1:
                        nc.sync.dma_start(_r(ao[hp][64:128, :]), _r(dst))

            # --- output projection ---
            with (
                tc.tile_pool(name="outp", bufs=3) as outp,
                tc.tile_pool(name="wopsum", bufs=2, space="PSUM") as wop,
            ):
                for t in range(NT):
                    po = wop.tile([128, D], F32, tag="po")
                    for nh in range(2):
                        ns = slice(nh * 512, (nh + 1) * 512)
                        for cc in range(2):
                            nc.tensor.matmul(
                                po[:, ns],
                                _r(ao[cc][:, t * 128:(t + 1) * 128]),
                                _r(wo[:, cc, ns]),
                                start=(cc == 0), stop=(cc == 1))
                    ot = outp.tile([128, D], F32, tag="ot")
                    nc.scalar.copy(ot[:], po[:])
                    nc.sync.dma_start(out_d[t * 128:(t + 1) * 128, :], ot[:])

    nc.compile()
    return nc


def _round_f32r(a):
    """Round fp32 to the fp32r grid (11-bit mantissa, round-to-nearest)."""
    bits = np.ascontiguousarray(a, np.float32).view(np.uint32)
    return ((bits + 0x800) & 0xFFFFF000).view(np.float32)


def make_in_maps(x, freqs_cos, freqs_sin, wq, wk, wv, wo):
    """Host-side sharding + layout prep. Returns per-core input dicts."""
    x = np.asarray(x, np.float32)
    fc = np.asarray(freqs_cos, np.float32)
    fs = np.asarray(freqs_sin, np.float32)
    wq = np.asarray(wq, np.float32)
    wk = np.asarray(wk, np.float32)
    wv = np.asarray(wv, np.float32)
    wo = np.asarray(wo, np.float32)

    perm = np.concatenate([np.arange(0, HD, 2), np.arange(1, HD, 2)])
    cosT = np.ascontiguousarray(fc.T)            # (32, T)
    sinT = np.ascontiguousarray(fs.T)
    cosf = np.concatenate([cosT] * 4, axis=0)    # (128, T)
    sinf = np.concatenate([-sinT, sinT, -sinT, sinT], axis=0)

    jj = np.arange(128)[:, None]
    ii = np.arange(512)[None, :]
    msk = np.stack(
        [np.where(r * 128 + jj <= ii, 0.0, -1e30) for r in range(4)], axis=0
    ).astype(np.float32)                         # (4, 128, 512)
    mskT = np.ascontiguousarray(msk.transpose(1, 0, 2))  # (128, 4, 512)

    in_maps = []
    for c in range(N_CORES):
        b, g = divmod(c, 4)
        wq_c = wq[:, g * QCOLS:(g + 1) * QCOLS]
        wq_c = np.ascontiguousarray(
            wq_c.reshape(D, NQH, HD)[:, :, perm].reshape(D, QCOLS))
        wk_c = np.ascontiguousarray(wk[:, g * HD:(g + 1) * HD][:, perm])
        wv_c = np.ascontiguousarray(wv[:, g * HD:(g + 1) * HD])
        wo_c = np.ascontiguousarray(wo[g * QCOLS:(g + 1) * QCOLS, :])
        xT_c = np.ascontiguousarray(x[b].T)
        in_maps.append({
            "xT": _round_f32r(xT_c), "wq": _round_f32r(wq_c),
            "wk": _round_f32r(wk_c), "wv": _round_f32r(wv_c),
            "wo": _round_f32r(wo_c),
            "cosf": cosf, "sinf": sinf, "msk": mskT,
            "onec": np.ones((128, HD), np.float32),
        })
    return in_maps


def run_on_cores(in_maps, trace=False, **kwargs):
    if "nc" not in _cache:
        _cache["nc"] = build_nc()
    return run_bass_kernel_spmd(
        _cache["nc"], in_maps, core_ids=list(range(N_CORES)), trace=trace,
        **kwargs)


def kernel(x, freqs_cos, freqs_sin, wq, wk, wv, wo):
    in_maps = make_in_maps(x, freqs_cos, freqs_sin, wq, wk, wv, wo)
    res = run_on_cores(in_maps)
    outs = [res.results[c]["out"] for c in range(N_CORES)]
    full = np.empty((B, T, D), np.float32)
    for b in range(B):
        full[b] = outs[4 * b] + outs[4 * b + 1] + outs[4 * b + 2] + outs[4 * b + 3]
    return full



# revision 5
# speedup vs baseline: 1.8877x; 1.8877x over previous
"""GQA attention kernel for Trainium2, 8 NeuronCores (v2, fp16).

Problem: B=2, T=2048, D=1024, 16 Q heads / 4 KV heads, head_dim=64, RoPE,
causal softmax, out-projection.

Sharding: 8 cores = 2 (batch) x 4 (KV group). Core c handles batch c//4 and
KV group g=c%4 (query heads 4g..4g+3). wq/wk/wv column-sharded, wo
row-sharded; the 4 partial outputs per batch are summed on the host.

v2 changes vs the fp32r baseline:
  * All matmul operands are fp16 (PSUM accumulation stays fp32).  fp32r
    matmuls measured ~3 cycles/row on HW and block LDWEIGHTS overlap (no
    FWL for fp32); fp16 runs 1 row/cycle with fast weight load.
  * Causal windows are shifted per key-block: for key block tj the query
    range is [128*tj, T) instead of 512-aligned chunks, trimming ~12% of
    score/PV rows, and the mask becomes a single static j<=i pattern
    applied with affine_select on the (otherwise idle) Pool engine after
    exp (multiplicative 0-fill on the first 128 columns only).
  * Queries processed in two 1024-halves; per (head, half, tj) ONE wide
    exp instruction covers the whole window (up to 1024 cols spanning two
    PSUM banks), halving ACT's ~350-cycle/instruction overhead count.
  * Softmax denominator: ones-columns ride in the PV stationary operand
    (col 0 and col 65 of v), so even heads get L at partition 64 and odd
    heads at partition 63 with pv rows on their natural ao partitions --
    no partition-base-shifting DMAs for ao assembly.
  * 1/L: L-row is staged to SBUF (fp16), DMA-transposed to [128, 8],
    reciprocal'd with full lane parallelism (the baseline burned 52us
    doing [1,512] reciprocals on one DVE lane), DMA'd back to a row and
    broadcast to 64 partitions with log2 doubling DMAs.
  * Projections for the second query half and the first half of the out
    projection are interleaved into the ACT-bound attention phases.
"""

import numpy as np
import sys

sys.path.insert(0, "/opt/trn_rl_repo")

from concourse import bass, bacc, mybir, tile  # noqa: E402
from concourse.bass_utils import run_bass_kernel_spmd  # noqa: E402

F32 = mybir.dt.float32
F16 = mybir.dt.float16

B, T, D = 2, 2048, 1024
HD = 64                      # head dim
NQH = 4                      # query heads per core
QCOLS = NQH * HD             # 256
KC = D // 128                # 8 contraction chunks
N_CORES = 8

_cache = {}


def _chunks512(a, b):
    """Split [a, b) at multiples of 512."""
    out = []
    while a < b:
        nxt = min(b, (a // 512 + 1) * 512)
        out.append((a, nxt))
        a = nxt
    return out


def build_nc():
    """Build the (SPMD-identical) single-core bass program."""
    nc = bacc.Bacc("TRN2", target_bir_lowering=False, debug=False)

    xT_d = nc.declare_dram_parameter("xT", [D, T], F16, isOutput=False)
    wq_d = nc.declare_dram_parameter("wq", [D, QCOLS], F16, isOutput=False)
    wk_d = nc.declare_dram_parameter("wk", [D, HD], F16, isOutput=False)
    wv_d = nc.declare_dram_parameter("wv", [D, HD], F16, isOutput=False)
    wo_d = nc.declare_dram_parameter("wo", [QCOLS, D], F16, isOutput=False)
    cos_d = nc.declare_dram_parameter("cosf", [128, T], F16, isOutput=False)
    sin_d = nc.declare_dram_parameter("sinf", [128, T], F16, isOutput=False)
    out_d = nc.declare_dram_parameter("out", [T, D], F32, isOutput=True)

    EXP = mybir.ActivationFunctionType.Exp

    with tile.TileContext(nc) as tc:
        with (
            tc.tile_pool(name="sb", bufs=1) as sb,
            tc.tile_pool(name="rotp", bufs=2) as rotp,
            tc.tile_pool(name="atp", bufs=3) as atp,
            tc.tile_pool(name="stgp", bufs=2) as stgp,
            tc.tile_pool(name="lcp", bufs=2) as lcp,
            tc.tile_pool(name="lip", bufs=2) as lip,
            tc.tile_pool(name="bcp", bufs=2) as bcp,
            tc.tile_pool(name="otp", bufs=3) as otp,
            tc.tile_pool(name="pp", bufs=2, space="PSUM") as pp,
            tc.tile_pool(name="scp", bufs=2, space="PSUM") as scp,
            tc.tile_pool(name="pvp", bufs=1, space="PSUM") as pvp,
        ):
            wq_s = sb.tile([128, KC, QCOLS], F16, tag="wq")
            wk_s = sb.tile([128, KC, HD], F16, tag="wk")
            wv_s = sb.tile([128, KC, HD], F16, tag="wv")
            wo_s = sb.tile([128, 2, D], F16, tag="wo")
            cosf = sb.tile([128, T], F16, tag="cosf")
            sinf = sb.tile([128, T], F16, tag="sinf")
            xTs = sb.tile([128, KC, T], F16, tag="xT")
            qT = [sb.tile([128, T], F16, tag=f"qT{hp}", name=f"qT{hp}")
                  for hp in range(2)]
            # kT duplicated into both partition halves so scores matmuls can
            # read it at base partition 0 (even heads) or 64 (odd heads).
            kT = sb.tile([128, T], F16, tag="kT")
            # v columns: 0 = ones, 1..64 = v channels, 65 = ones.  Even heads
            # use cols 1:66 (L lands at out-partition 64); odd heads use cols
            # 0:65 at out base 63 (L at 63, pv at 64:128).
            v = sb.tile([128, 16, HD + 2], F16, tag="v")
            ao = [sb.tile([128, T], F16, tag=f"ao{hp}", name=f"ao{hp}")
                  for hp in range(2)]

            for k in range(KC):
                nc.sync.dma_start(wq_s[:, k, :], wq_d[k * 128:(k + 1) * 128, :])
                nc.sync.dma_start(wk_s[:, k, :], wk_d[k * 128:(k + 1) * 128, :])
                nc.sync.dma_start(wv_s[:, k, :], wv_d[k * 128:(k + 1) * 128, :])
                nc.sync.dma_start(xTs[:, k, :], xT_d[k * 128:(k + 1) * 128, :])
            nc.sync.dma_start(cosf[:], cos_d[:])
            nc.sync.dma_start(sinf[:], sin_d[:])
            for c in range(2):
                nc.sync.dma_start(wo_s[:, c, :], wo_d[c * 128:(c + 1) * 128, :])
            nc.gpsimd.memset(v[:, :, 0:1], 1.0)
            nc.gpsimd.memset(v[:, :, HD + 1:HD + 2], 1.0)

            def proj_chunk(ci):
                cs = slice(ci * 512, (ci + 1) * 512)
                pk = pp.tile([128, 512], F32, tag="proj", name=f"pk{ci}")
                for k in range(KC):
                    nc.tensor.matmul(pk[0:64, :], wk_s[:, k, :], xTs[:, k, cs],
                                     start=(k == 0), stop=(k == KC - 1))
                nc.vector.tensor_copy(kT[0:64, cs], pk[0:64, :])
                for hp in range(2):
                    pq = pp.tile([128, 512], F32, tag="proj",
                                 name=f"pq{ci}{hp}")
                    for k in range(KC):
                        nc.tensor.matmul(
                            pq[:, :], wq_s[:, k, hp * 128:(hp + 1) * 128],
                            xTs[:, k, cs], start=(k == 0), stop=(k == KC - 1))
                    nc.vector.tensor_copy(qT[hp][:, cs], pq[:, :])
                    for t in (4 * ci + 2 * hp, 4 * ci + 2 * hp + 1):
                        pvt = pp.tile([128, 512], F32, tag="proj",
                                      name=f"pvt{t}")
                        for k in range(KC):
                            nc.tensor.matmul(
                                pvt[:, 0:HD],
                                xTs[:, k, t * 128:(t + 1) * 128],
                                wv_s[:, k, :],
                                start=(k == 0), stop=(k == KC - 1))
                        nc.vector.tensor_copy(v[:, t, 1:HD + 1], pvt[:, 0:HD])

            def rope(dst, nrows, cs):
                """dst = dst*cos + rot_half(dst)*sin on columns cs."""
                w = cs.stop - cs.start
                rot = rotp.tile([128, 1024], F16, tag="rot", name="rot")
                for blk in range(nrows // 64):
                    r0 = blk * 64
                    nc.sync.dma_start(rot[r0:r0 + 32, 0:w],
                                      dst[r0 + 32:r0 + 64, cs])
                    nc.sync.dma_start(rot[r0 + 32:r0 + 64, 0:w],
                                      dst[r0:r0 + 32, cs])
                nc.vector.tensor_mul(dst[0:nrows, cs], dst[0:nrows, cs],
                                     cosf[0:nrows, cs])
                nc.vector.tensor_mul(rot[0:nrows, 0:w], rot[0:nrows, 0:w],
                                     sinf[0:nrows, cs])
                nc.vector.tensor_add(dst[0:nrows, cs], dst[0:nrows, cs],
                                     rot[0:nrows, 0:w])

            def rope_half(qh):
                cs = slice(qh * 1024, (qh + 1) * 1024)
                rope(qT[0], 128, cs)
                rope(qT[1], 128, cs)
                rope(kT, 64, cs)
                nc.sync.dma_start(kT[64:128, cs], kT[0:64, cs])

            def attn_head(qh, h):
                hp, hr = divmod(h, 2)
                qrow = slice(64 * hr, 64 * hr + 64)
                prow = slice(0, 65)
                vcols = slice(1, HD + 2)
                pv = pvp.tile([128, 1024], F32, tag="pv", name=f"pv{qh}{h}")

                def emit_pv(tj, ws, W, at):
                    lo = ws - 1024 * qh
                    for (a, b) in _chunks512(lo, lo + W):
                        bk = a // 512
                        nc.tensor.matmul(
                            pv[prow, a:b], v[:, tj, vcols],
                            at[:, a - lo:b - lo],
                            start=(tj == 0),
                            stop=(tj == 8 * qh + 4 * bk + 3))

                prev = None
                for tj in range(8 * (qh + 1)):
                    ws = max(1024 * qh, 128 * tj)
                    W = 1024 * (qh + 1) - ws
                    sc = scp.tile([128, 1024], F32, tag="sc",
                                  name=f"sc{qh}{h}{tj}")
                    for (a, b) in _chunks512(0, W):
                        nc.tensor.matmul(
                            sc[:, a:b], kT[qrow, tj * 128:(tj + 1) * 128],
                            qT[hp][qrow, ws + a:ws + b],
                            start=True, stop=True)
                    at = atp.tile([128, 1024], F16, tag="at",
                                  name=f"at{qh}{h}{tj}")
                    nc.scalar.activation(at[:, 0:W], sc[:, 0:W], EXP,
                                         scale=0.125)
                    if ws == 128 * tj:  # window starts at the diagonal
                        nc.gpsimd.affine_select(
                            at[:, 0:128], at[:, 0:128],
                            pattern=[[1, 128]],
                            compare_op=mybir.AluOpType.is_ge,
                            fill=0.0, base=0, channel_multiplier=-1)
                    if prev is not None:
                        emit_pv(*prev)
                    prev = (tj, ws, W, at)
                emit_pv(*prev)

                # normalize: stage pv (+L row) to SBUF fp16, transpose the L
                # row via DMA, reciprocal on 128 lanes, broadcast back.
                # PE outputs must start at partition 0/32/64, so all heads
                # compute at base 0; odd heads DMA the normalized result to
                # ao partitions 64:128 afterwards.
                stg = stgp.tile([128, 1024], F16, tag="stg", name=f"st{qh}{h}")
                nc.vector.tensor_copy(stg[prow, :], pv[prow, :])
                lc = lcp.tile([128, 8], F16, tag="lc", name=f"lc{qh}{h}")
                nc.sync.dma_start(lc[:, :], stg[64:65, :])
                li = lip.tile([128, 8], F16, tag="li", name=f"li{qh}{h}")
                with nc.allow_low_precision(reason="fp16 1/L"):
                    nc.vector.reciprocal(li[:, :], lc[:, :])
                bc = bcp.tile([64, 1024], F16, tag="bc", name=f"bc{qh}{h}")
                nc.sync.dma_start(bc[0:1, :], li[:, :])
                for dd in (1, 2, 4, 8, 16, 32):
                    nc.sync.dma_start(bc[dd:2 * dd, :], bc[0:dd, :])
                half = slice(1024 * qh, 1024 * (qh + 1))
                if hr == 0:
                    nc.vector.tensor_mul(ao[hp][0:64, half],
                                         stg[0:64, :], bc[0:64, :])
                else:
                    aot = stgp.tile([128, 1024], F16, tag="aot",
                                    name=f"aot{qh}{h}")
                    nc.vector.tensor_mul(aot[0:64, :], stg[0:64, :],
                                         bc[0:64, :])
                    nc.sync.dma_start(ao[hp][64:128, half], aot[0:64, :])

            def outproj_tile(t, eng):
                for nh in range(2):
                    po = pp.tile([128, 512], F32, tag="proj",
                                 name=f"po{t}{nh}")
                    for cc in range(2):
                        nc.tensor.matmul(
                            po[:, :], ao[cc][:, t * 128:(t + 1) * 128],
                            wo_s[:, cc, nh * 512:(nh + 1) * 512],
                            start=(cc == 0), stop=(cc == 1))
                    ot = otp.tile([128, 512], F32, tag="ot", name=f"ot{t}{nh}")
                    if eng == "v":
                        nc.vector.tensor_copy(ot[:, :], po[:, :])
                    else:
                        nc.scalar.copy(ot[:, :], po[:, :])
                    nc.sync.dma_start(
                        out_d[t * 128:(t + 1) * 128, nh * 512:(nh + 1) * 512],
                        ot[:, :])

            # ---- schedule ----
            proj_chunk(0)
            proj_chunk(1)
            rope_half(0)
            attn_head(0, 0)
            proj_chunk(2)
            attn_head(0, 1)
            proj_chunk(3)
            attn_head(0, 2)
            rope_half(1)
            attn_head(0, 3)
            for h in range(4):
                attn_head(1, h)
                outproj_tile(2 * h, "v")
                outproj_tile(2 * h + 1, "v")
            for t in range(8, 16):
                outproj_tile(t, "v" if t % 2 == 0 else "s")

    nc.compile()
    return nc


def make_in_maps(x, freqs_cos, freqs_sin, wq, wk, wv, wo):
    """Host-side sharding + layout prep. Returns per-core input dicts."""
    x = np.asarray(x, np.float32)
    fc = np.asarray(freqs_cos, np.float32)
    fs = np.asarray(freqs_sin, np.float32)
    wq = np.asarray(wq, np.float32)
    wk = np.asarray(wk, np.float32)
    wv = np.asarray(wv, np.float32)
    wo = np.asarray(wo, np.float32)

    perm = np.concatenate([np.arange(0, HD, 2), np.arange(1, HD, 2)])
    cosT = np.ascontiguousarray(fc.T)            # (32, T)
    sinT = np.ascontiguousarray(fs.T)
    cosf = np.concatenate([cosT] * 4, axis=0).astype(np.float16)
    sinf = np.concatenate([-sinT, sinT, -sinT, sinT], axis=0).astype(np.float16)

    in_maps = []
    for c in range(N_CORES):
        b, g = divmod(c, 4)
        wq_c = wq[:, g * QCOLS:(g + 1) * QCOLS]
        wq_c = np.ascontiguousarray(
            wq_c.reshape(D, NQH, HD)[:, :, perm].reshape(D, QCOLS))
        wk_c = np.ascontiguousarray(wk[:, g * HD:(g + 1) * HD][:, perm])
        wv_c = np.ascontiguousarray(wv[:, g * HD:(g + 1) * HD])
        wo_c = np.ascontiguousarray(wo[g * QCOLS:(g + 1) * QCOLS, :])
        xT_c = np.ascontiguousarray(x[b].T)
        in_maps.append({
            "xT": xT_c.astype(np.float16), "wq": wq_c.astype(np.float16),
            "wk": wk_c.astype(np.float16), "wv": wv_c.astype(np.float16),
            "wo": wo_c.astype(np.float16),
            "cosf": cosf, "sinf": sinf,
        })
    return in_maps


def run_on_cores(in_maps, trace=False, **kwargs):
    if "nc" not in _cache:
        _cache["nc"] = build_nc()
    return run_bass_kernel_spmd(
        _cache["nc"], in_maps, core_ids=list(range(N_CORES)), trace=trace,
        **kwargs)


def kernel(x, freqs_cos, freqs_sin, wq, wk, wv, wo):
    in_maps = make_in_maps(x, freqs_cos, freqs_sin, wq, wk, wv, wo)
    res = run_on_cores(in_maps)
    outs = [res.results[c]["out"] for c in range(N_CORES)]
    full = np.empty((B, T, D), np.float32)
    for b in range(B):
        full[b] = outs[4 * b] + outs[4 * b + 1] + outs[4 * b + 2] + outs[4 * b + 3]
    return full
